# revision 23
# baseline (speedup 1.0000x reference)
"""Trainium2 Bass kernel for blocked-DCT high-frequency extractor.

Computes, for x (64, 3, 512, 512) f32:
  gray = 0.299*R + 0.587*G + 0.114*B                     (B,1,H,W)
  per 8x8 block:  Y = mask * (D @ block @ D.T)           (2D DCT + high-pass)
  output (64, 1, 512, 512) f32

Strategy (pure data parallel over batch, 8 batches/core on 8 cores; the
kernel is HBM-bound: 24 MiB in + 8 MiB out per core; DMA-union busy on
HW measures ~94.4 us at ~355 GB/s = the per-NC HBM cap, so the span
target is startup + ~94.4 us + a short drain).

Per core, per (batch, 128-row chunk) of the image:
  1. One fused 768 KB DMA on the SP HWDGE queue brings all 3 channel
     chunks into a (128h, 3*512w) tile (2 KB contiguous runs).
  2. Grayscale spread over three engines so none saturates:
     g0 = x0*(w0/w2) + x2 on DVE (scalar_tensor_tensor),
     gs = x1*(w1/w2) on ACT, g1 = g0 + gs on GpSimd.
  3. H-direction DCT: one f32 matmul with w2 * (I_16 kron D^T).
     Plain f32 matmuls run at 1/4 PE rate but read SBUF at only ~300
     GB/s -- measured: f32r matmuls (1 cycle/row, ~1.2 TB/s SBUF read
     bursts) stall concurrent input-DMA SBUF writes by ~46%, which on
     this DMA-roofline kernel costs more than the PE time they save.
     So mm1 (the big one) deliberately stays on the slow dtype.
  4. ACT drains PSUM -> SBUF with an f32 -> bf16 round (stream
     transpose cannot convert dtypes).
  5. DVE 32x32-block stream transpose. Because 8 | 32, this puts the
     intra-block w index on partitions.
  6. W-direction DCT *and* high-pass mask as two bf16 matmuls over the
     two strided free-column groups f%8<4 / f%8>=4: the mask only
     depends on (out-partition % 8 < 4) && (free % 8 < 4), so the f<4
     group uses weights whose columns are pre-scaled by the 0/1 mask
     vector and the f>=4 group uses plain weights. bf16 keeps the PE
     read burst at 600 GB/s for only ~0.6 us/chunk (vs f32 1.12 us at
     300), so the total burst-bytes crowding the DMA actually drop.
  7. DVE stream-transpose back, f32 PSUM -> SBUF -> natural layout.
  8. 256 KB contiguous output DMA on the ACT HWDGE queue (separate
     queue from the SP input stream).
"""

import os

import numpy as np

import concourse.bacc as bacc
import concourse.mybir as mybir
import concourse.tile as tile
from concourse.bass_utils import run_bass_kernel_spmd

N_CORES = 8
B, C, H, W = 64, 3, 512, 512
BLOC = B // N_CORES  # batches per core
P = 128              # SBUF partitions / chunk height
NCH = H // P         # 128-row chunks per image
F32 = mybir.dt.float32
BF16 = mybir.dt.bfloat16
GRAY_W = (0.299, 0.587, 0.114)

_NC = None          # cached compiled Bass module
LAST_RUN = None     # BassKernelResults of the most recent run (for test.py)


def _build_bass():
    nc = bacc.Bacc(
        "TRN2",
        target_bir_lowering=False,
        debug=False,
        num_devices=N_CORES,
    )
    x = nc.declare_dram_parameter("x", [BLOC, C, H, W], F32, isOutput=False)
    wts = nc.declare_dram_parameter("wts", [P, P], F32, isOutput=False)
    # bf16 weights for the W-direction matmul: [mask-scaled K | plain K]
    wtsb = nc.declare_dram_parameter("wtsb", [P, 2 * P], BF16, isOutput=False)
    out = nc.declare_dram_parameter("out", [BLOC, 1, H, W], F32, isOutput=True)

    ga = GRAY_W[0] / GRAY_W[2]
    gb = GRAY_W[1] / GRAY_W[2]
    mult = mybir.AluOpType.mult
    add = mybir.AluOpType.add

    with tile.TileContext(nc) as tc:
        with (
            tc.tile_pool(name="consts", bufs=1) as consts,
            tc.tile_pool(name="xin", bufs=4) as xin,
            tc.tile_pool(name="work", bufs=6) as work,
            tc.tile_pool(name="psum", bufs=4, space="PSUM") as psum_pool,
        ):
            # consts ride the ACT HWDGE queue so the SP queue opens with
            # the first 768 KB input chunk.
            wd = consts.tile([P, P], F32, tag="wd")
            nc.scalar.dma_start(wd[:], wts[:])
            wdb = consts.tile([P, 2 * P], BF16, tag="wdb")
            nc.scalar.dma_start(wdb[:], wtsb[:])

            # out-DMA for chunk i is emitted at the top of iteration i+1 so
            # ACT's in-order stream never parks on the wait for DVE's final
            # transpose ahead of the next chunk's compute ops.
            pending = None
            for b in range(BLOC):
                for hc in range(NCH):
                    hs = hc * P
                    # one 768 KB DMA: channels side by side in the free dim
                    xt = xin.tile([P, C * W], F32, tag="x")
                    xsrc = x[b].rearrange("c (n p) w -> n p c w", p=P)[hc]
                    nc.sync.dma_start(
                        xt[:].rearrange("p (c w) -> p c w", w=W), xsrc
                    )
                    x0 = xt[:, 0 * W:1 * W]
                    x1 = xt[:, 1 * W:2 * W]
                    x2 = xt[:, 2 * W:3 * W]
                    # grayscale split across DVE / ACT / Pool so no engine saturates
                    g0 = work.tile([P, W], F32, tag="g0")
                    nc.vector.scalar_tensor_tensor(g0[:], x0, ga, x2, mult, add)
                    gs = work.tile([P, W], F32, tag="gs")
                    nc.scalar.mul(gs[:], x1, gb)
                    # delayed out-DMA sits after the gray mul in ACT's
                    # in-order stream: the mul's dep (input DMA) lands much
                    # earlier than the DMA's dep (prev chunk's transpose)
                    if pending is not None:
                        nc.scalar.dma_start(*pending)
                    g1 = work.tile([P, W], F32, tag="g1")
                    nc.gpsimd.tensor_tensor(g1[:], gs[:], g0[:], add)
                    # H-direction DCT
                    p1 = psum_pool.tile([P, W], F32, tag="p1")
                    nc.tensor.matmul(p1[:], wd[:], g1[:], start=True, stop=True)
                    # PSUM -> SBUF with bf16 rounding on ACT
                    s1b = work.tile([P, W], BF16, tag="s1b")
                    nc.scalar.copy(s1b[:], p1[:])
                    # 32x32 block transpose (2-byte)
                    s1t = work.tile([P, W], BF16, tag="s1t")
                    nc.vector.transpose(s1t[:], s1b[:])
                    # W-direction DCT + mask: two bf16 matmuls over the
                    # strided free-column split
                    p2 = psum_pool.tile([P, W], F32, tag="p2")
                    p2v = p2[:].rearrange("p (g u) -> p g u", u=8)
                    s1v = s1t[:].rearrange("p (g u) -> p g u", u=8)
                    nc.tensor.matmul(
                        p2v[:, :, 0:4], wdb[:, 0:P], s1v[:, :, 0:4],
                        start=True, stop=True,
                    )
                    nc.tensor.matmul(
                        p2v[:, :, 4:8], wdb[:, P:2 * P], s1v[:, :, 4:8],
                        start=True, stop=True,
                    )
                    # block transpose back to natural layout, PSUM -> SBUF
                    s2t = work.tile([P, W], F32, tag="s2t", bufs=8)
                    nc.vector.transpose(s2t[:], p2[:])
                    # outputs ride the ACT HWDGE queue; inputs own the SP queue
                    pending = (out[b, 0, hs:hs + P, :], s2t[:])
            nc.scalar.dma_start(*pending)
    nc.compile()
    return nc


def _host_constants(dct_matrix, mask):
    D = np.asarray(dct_matrix, dtype=np.float32)
    M = np.asarray(mask, dtype=np.float32)
    kron = np.kron(np.eye(P // 8, dtype=np.float32), D.T).astype(np.float32)
    # mm1 weight carries the full grayscale scale w2 (mm2 is unscaled)
    wts = (np.float32(GRAY_W[2]) * kron).astype(np.float32)
    # mm2 weights: mask zeroes (out-partition%8 < 4) only for the f%8<4
    # free columns -> scale the masked copy's columns by M[0, i%8]
    pi = np.arange(P)
    mvec = np.ascontiguousarray(M[0, pi % 8], dtype=np.float32)
    wtsb = np.concatenate([kron * mvec[None, :], kron], axis=1).astype(
        mybir.dt.np(BF16)
    )
    return wts, wtsb


def kernel(x, dct_matrix, mask):
    global _NC, LAST_RUN
    x = np.ascontiguousarray(np.asarray(x, dtype=np.float32))
    assert x.shape == (B, C, H, W)
    wts, wtsb = _host_constants(dct_matrix, mask)

    if _NC is None:
        _NC = _build_bass()

    in_maps = [
        {"x": np.ascontiguousarray(x[i * BLOC:(i + 1) * BLOC]),
         "wts": wts, "wtsb": wtsb}
        for i in range(N_CORES)
    ]
    trace = bool(int(os.environ.get("DCT_TRACE", "0")))
    LAST_RUN = run_bass_kernel_spmd(
        _NC, in_maps, list(range(N_CORES)), trace=trace,
    )
    out = np.concatenate([LAST_RUN.results[i]["out"] for i in range(N_CORES)], axis=0)
    return out
